# revision 45
# baseline (speedup 1.0000x reference)
"""Multi-head attention (B=2, S=2048, D=1024, H=16) on 8 NeuronCores.

Sharding: core c handles batch b = c//4 and 4 heads starting at (c%4)*4
(data parallel over batch x tensor parallel over heads; wQ/wK/wV split
column-wise by head, wO row-wise; partial outputs summed on host).

v2: single fused pipeline, engines balanced against the cost model:
  - Projections in fp8 hi-lo DoubleRow form: x = x_hi + x_lo and w*64 =
    w_hi + w_lo (both fp8e4, host-prepared; the *64 keeps w out of fp8's
    subnormal range), accumulate x_hi*w_hi + x_hi*w_lo + x_lo*w_hi in psum
    (the dropped lo*lo term is ~2^-8 relative), scale 1/64 + bias applied at
    copyback. DoubleRow contracts 256 model-dims per matmul at 0.5 cyc/row,
    so a projection costs 0.75x its bf16 form in PE time at bf16-level
    accuracy.
  - Projections stream in 512-query-column chunks interleaved with the mask
    DMAs so the PE starts ~4us in: Q[0:1024] first (ACT copyback while ACT is
    idle), then K, V, Q[1024:2048] (DVE copybacks once the exp stream owns
    ACT).
  - Attention per (s1c query-1024-block, head): scoresT psum [128,1024]
    (2 bf16 matmuls), exp on ACT (psum -> sbuf bf16), {0,1}-mask multiply on
    DVE (bf16 2x mode), PV into two [128,512] psum chains. Each head's V4x
    block is [64 ones-columns | 64 V-columns], so the PV matmul lands the
    softmax denominator REPLICATED on psum partitions 0-63 and ctx on
    64-127 (out-free-size-only matmul cost makes the wide ones block free).
  - Normalization per chain: one 64-wide reciprocal_approx_fast at partition
    base 0 (custom-DVE ops only honor base 0 on real HW) straight from psum,
    then one multiply (mixed 32-aligned operand bases are legal). Odd heads
    restack to partitions 64-127 via a small batched SBUF-to-SBUF DMA.
  - PV for kc is emitted D kc-steps late so the PE queue never head-blocks
    on the exp chain of the same kc; the DMA issue order is hand-sequenced
    (K fully first: exp needs only scores, so the exp stream unblocks early
    while mask-muls and PV lag in the et pool until masks/V land).
  - Output projection per query half into [128,512] psums; copyback on DVE
    for s1c=0 (ACT is exp-saturated), alternating DVE/ACT in the tail for
    s1c=1; 256-row quarters ship as single batched DMAs. s1c=1 processes
    heads 0,1,3,2 so the final head needs no restack DMA on the tail path.
  - PSUM: scores 2x[128,1024] (4 banks) + PV 2x[128,512] (2) + proj/outproj
    2x[128,512] (2) = 8 banks; the tail outproj also borrows the idle
    scores pool for a 4-deep ring.

The wV bias is dropped on device: softmax rows sum to 1, so it contributes
exactly wV_b @ wO_w.T, folded into the host-side bias add. No max-subtraction
in softmax (scores are O(5), bf16 exp cannot overflow).

Host: out[b] = sum of the 4 cores' partials + (wO_b + wV_b @ wO_w.T).
"""

import numpy as np
from contextlib import ExitStack

import concourse.bacc as bacc
import concourse.tile as tile
from concourse import mybir
import ml_dtypes

B, S, DM, H, DK = 2, 2048, 1024, 16, 64
NCORES = 8
GROUPS = 4          # cores per batch
HPC = H // GROUPS   # heads per core = 4
P = 128
KT = DM // P        # 8 k-tiles over the model dim
CW = HPC * DK       # projected width per core = 256
SCALE = 1.0 / np.sqrt(DK)
WS = 64.0           # fp8 weight pre-scale (power of 2; undone at copyback)

MM_DT = mybir.dt.bfloat16
MM_NP = ml_dtypes.bfloat16
F8_DT = mybir.dt.float8e4
F8_NP = ml_dtypes.float8_e4m3

F32 = mybir.dt.float32
EXPF = mybir.ActivationFunctionType.Exp
IDENT = mybir.ActivationFunctionType.Identity
DR = mybir.MatmulPerfMode.DoubleRow

_cache: dict = {}


def _build():
    nc = bacc.Bacc("TRN2", target_bir_lowering=False, debug=False)

    xs = {}
    for nm in ("q", "k", "v"):
        for part in ("hi", "lo"):
            xs[(nm, part)] = nc.dram_tensor(f"x{nm}_{part}", [DM, S], F8_DT,
                                            kind="ExternalInput")
    ws = {}
    for nm in ("q", "k", "v"):
        for part in ("hi", "lo"):
            ws[(nm, part)] = nc.dram_tensor(f"w{nm}_{part}", [DM, CW], F8_DT,
                                            kind="ExternalInput")
    wqb = nc.dram_tensor("wqb", [CW, 1], F32, kind="ExternalInput")
    wkb = nc.dram_tensor("wkb", [CW, 1], F32, kind="ExternalInput")
    woT = nc.dram_tensor("woT", [CW, DM], MM_DT, kind="ExternalInput")
    maskT = nc.dram_tensor("maskT", [S, S], MM_DT, kind="ExternalInput")
    out = nc.dram_tensor("out", [S, DM], MM_DT, kind="ExternalOutput")

    with tile.TileContext(nc) as tc, ExitStack() as ctx:
        const = ctx.enter_context(tc.tile_pool(name="const", bufs=1))
        big = ctx.enter_context(tc.tile_pool(name="big", bufs=1))
        xpool = ctx.enter_context(tc.tile_pool(name="xpool", bufs=2))
        mpool = ctx.enter_context(tc.tile_pool(name="mpool", bufs=1))
        epool = ctx.enter_context(tc.tile_pool(name="epool", bufs=21))
        rpool = ctx.enter_context(tc.tile_pool(name="rpool", bufs=3))
        opool = ctx.enter_context(tc.tile_pool(name="opool", bufs=2))
        pp_sc = ctx.enter_context(tc.tile_pool(name="pp_sc", bufs=2, space="PSUM"))
        pp_ctx = ctx.enter_context(tc.tile_pool(name="pp_ctx", bufs=2, space="PSUM"))
        pp_pj = ctx.enter_context(tc.tile_pool(name="pp_pj", bufs=2, space="PSUM"))

        # persistent activations
        # Q4T/K4T: [dk-in-pair (part), pair c, s]; head h = c*2 + (p//64)
        Q4T = big.tile([P, 2, S], MM_DT, name="Q4T")
        K4T = big.tile([P, 2, S], MM_DT, name="K4T")
        V4x = big.tile([P, 16, HPC * 2 * DK], MM_DT, name="V4x")
        ctxT = [[big.tile([P, 1024], MM_DT, name=f"ctxT{i}_{j}") for j in range(2)]
                for i in range(2)]

        # 64 ones columns per head (cols h*128..h*128+63): the PV matmul then
        # lands the softmax denominator replicated on psum partitions 0-63
        # (reciprocal_approx_fast only honors partition base 0 on HW), and
        # ctx on partitions 64-127.
        for h in range(HPC):
            nc.vector.memset(V4x[:, :, h * 128 : h * 128 + 64], 1.0)

        w_sbs = {}
        b_sbs = {}

        def load_w(nm, bd):
            for part in ("hi", "lo"):
                w_sb = const.tile([P, KT, CW], F8_DT, name=f"w{nm}{part}_sb")
                nc.sync.dma_start(
                    out=w_sb[:],
                    in_=ws[(nm, part)].ap().rearrange("(t p) m -> p t m", p=P))
                w_sbs[(nm, part)] = w_sb
            if bd is not None:
                b_sb = const.tile([P, 2], F32, name=f"b{nm}_sb")
                nc.sync.dma_start(
                    out=b_sb[:], in_=bd.ap().rearrange("(m p) o -> p (m o)", p=P))
                b_sbs[nm] = b_sb

        x_tiles = {}
        def load_x(nm, sc, bufs=2):
            """Issue DMA for x columns [sc*512, (sc+1)*512] (hi+lo)."""
            ts = []
            for part in ("hi", "lo"):
                t = xpool.tile([P, KT, 512], F8_DT, tag=f"x{nm}{part}", bufs=bufs)
                nc.sync.dma_start(
                    out=t[:],
                    in_=xs[(nm, part)].ap().rearrange("(t p) s -> p t s", p=P)
                    [:, :, sc * 512:(sc + 1) * 512])
                ts.append(t)
            x_tiles[(nm, sc)] = ts

        def proj_qk(nm, dst, sc, eng):
            """Project x cols [sc*512,(sc+1)*512] -> dst[:, :, cols] (+bias)."""
            xh, xl = x_tiles.pop((nm, sc))
            wh, wl = w_sbs[(nm, "hi")], w_sbs[(nm, "lo")]
            b_sb = b_sbs[nm]
            for m in range(2):
                ps = pp_pj.tile([P, 512], F32, tag="pj", name="ps_pj")
                i = 0
                for wt, xt in ((wh, xh), (wh, xl), (wl, xh)):
                    for c in range(4):
                        nc.tensor.matmul(
                            ps[:], wt[:, 2 * c:2 * c + 2, m * 128:(m + 1) * 128],
                            xt[:, 2 * c:2 * c + 2, :],
                            start=(i == 0), stop=(i == 11), perf_mode=DR)
                        i += 1
                dcols = dst[:, m, sc * 512:(sc + 1) * 512]
                if eng == "act":
                    nc.scalar.activation(dcols, ps[:], IDENT,
                                         bias=b_sb[:, m:m + 1], scale=1.0 / WS)
                else:
                    nc.vector.tensor_scalar(
                        out=dcols, in0=ps[:], scalar1=1.0 / WS,
                        scalar2=b_sb[:, m:m + 1],
                        op0=mybir.AluOpType.mult, op1=mybir.AluOpType.add)

        def proj_v(sc):
            """Project v cols [sc*512,(sc+1)*512] (4 s-tiles of 128)."""
            xh, xl = x_tiles.pop(("v", sc))
            wh, wl = w_sbs[("v", "hi")], w_sbs[("v", "lo")]
            for st in range(4):
                ps = pp_pj.tile([P, 512], F32, tag="pj", name="ps_pj")
                i = 0
                for xt, wt in ((xh, wh), (xh, wl), (xl, wh)):
                    for c in range(4):
                        nc.tensor.matmul(
                            ps[:, 0:CW],
                            xt[:, 2 * c:2 * c + 2, st * 128:(st + 1) * 128],
                            wt[:, 2 * c:2 * c + 2, :],
                            start=(i == 0), stop=(i == 11), perf_mode=DR)
                        i += 1
                sidx = sc * 4 + st
                nc.vector.tensor_scalar(
                    out=V4x.rearrange("p s (h e) -> p s h e", e=2 * DK)[:, sidx, :, DK:2 * DK],
                    in0=ps[:, 0:CW].rearrange("p (h e) -> p h e", e=DK),
                    scalar1=1.0 / WS, scalar2=None, op0=mybir.AluOpType.mult)

        # mask chunks: [(s1c, mc)] covers kc in [mc*4, mc*4+4) for query block s1c
        mask_sb = {}
        def load_mask(s1c, mc):
            mt = mpool.tile([P, 4, 1024], MM_DT, tag=f"m{mc}")
            nc.sync.dma_start(
                out=mt[:],
                in_=maskT.ap().rearrange("(t p) s -> p t s", p=P)
                [:, mc * 4:(mc + 1) * 4, s1c * 1024:(s1c + 1) * 1024])
            mask_sb[(s1c, mc)] = mt

        # ---------------- lead-in: DMA issue order + Q/K projections --------
        # DMA device is serialized; this issue order is the delivery order.
        # K ships first and completely: exp needs only scores (K x Q), so the
        # whole exp stream unblocks early; mask-muls and PV lag behind in the
        # et pool until the masks/V land.
        load_w("k", wkb)
        load_x("k", 0, bufs=3)
        load_w("q", wqb)
        load_x("q", 0)
        load_x("q", 1)
        load_x("k", 1, bufs=3)
        load_x("k", 2, bufs=3)
        load_x("k", 3, bufs=3)
        load_w("v", None)
        load_mask(0, 0)
        load_x("v", 0)
        load_mask(0, 1)
        load_x("v", 1)
        load_mask(0, 2)
        load_x("v", 2)
        load_mask(0, 3)
        load_x("v", 3)
        woT_sb = const.tile([P, 2, DM], MM_DT, name="woT_sb")

        proj_qk("k", K4T, 0, "act")
        proj_qk("q", Q4T, 0, "act")
        proj_qk("q", Q4T, 1, "act")
        for sc in range(1, 4):
            proj_qk("k", K4T, sc, "dve")

        # ---------------- attention ----------------
        def outproj(s1c, qh, eng, pools=None):
            """Output projection for query rows [s1c*1024 + qh*512 ...+512].

            Copybacks collect into [P, 2, 2, 512] staging tiles so each
            256-row quarter ships as a single DMA (HWDGE issue overhead
            dominates these short transfers). `pools` can alternate psum
            pools (the scores pool is idle in the tail)."""
            if pools is None:
                pools = [(pp_pj, "pj")]
            pi = 0
            for sb in range(2):
                ob = opool.tile([P, 2, 2, 512], MM_DT, tag="ob")
                for st in range(2):
                    q0 = qh * 512 + (sb * 2 + st) * 128
                    for n2 in range(2):
                        pool, ptag = pools[pi % len(pools)]
                        pi += 1
                        ps = pool.tile([P, 512], F32, tag=ptag, name="ps_pj")
                        for c2 in range(2):
                            nc.tensor.matmul(
                                ps[:], ctxT[c2][s1c][:, q0:q0 + 128],
                                woT_sb[:, c2, n2 * 512:(n2 + 1) * 512],
                                start=(c2 == 0), stop=(c2 == 1))
                        e = eng if eng != "mix" else ("act" if (st + n2) % 2 else "dve")
                        if e == "act":
                            nc.scalar.activation(ob[:, st, n2, :], ps[:], IDENT)
                        else:
                            nc.vector.tensor_copy(out=ob[:, st, n2, :], in_=ps[:])
                row = s1c * 1024 + qh * 512 + sb * 256
                nc.sync.dma_start(
                    out=out.ap()[row:row + 256, :]
                    .rearrange("(t p) (n2 n) -> p t n2 n", p=P, n2=2),
                    in_=ob[:])

        def head(s1c, h, extra):
            """One head's scoresT/exp/mask/PV stream for query block s1c.

            `extra` maps kc -> thunk emitted after that kc's scores/exp/mask
            (projection or outproj work interleaved into this head's stream).
            PV for kc is emitted D steps late so the PE queue never
            head-blocks on the exp+mask chain of the same kc.
            """
            D = 5 if extra else 3
            base = (h % 2) * 64
            c = h // 2
            cps = [pp_ctx.tile([P, 512], F32, tag="cps", name=f"cps{i}")
                   for i in range(2)]

            def pv(kc):
                for qh in range(2):
                    nc.tensor.matmul(
                        cps[qh][:], V4x[:, kc, h * 128:(h + 1) * 128],
                        ets[kc][:, qh * 512:(qh + 1) * 512],
                        start=(kc == 0), stop=(kc == 15))

            ets = {}
            for kc in range(16):
                # PV first: it is ready (its et landed D tiles ago), so it
                # fills the PE queue while the scores matmul below waits for
                # its psum buffer to be drained by ACT.
                if kc >= D:
                    pv(kc - D)
                ps = pp_sc.tile([P, 1024], F32, tag="sc", name="ps_sc")
                lhsT = K4T[base:base + 64, c, kc * 128:(kc + 1) * 128]
                for n2 in range(2):
                    col = s1c * 1024 + n2 * 512
                    nc.tensor.matmul(
                        ps[:, n2 * 512:(n2 + 1) * 512], lhsT,
                        Q4T[base:base + 64, c, col:col + 512],
                        start=True, stop=True)
                et = epool.tile([P, 1024], MM_DT, tag="et")
                nc.scalar.activation(et[:], ps[:], EXPF)
                nc.vector.tensor_mul(
                    et[:], et[:], mask_sb[(s1c, kc // 4)][:, kc % 4, :])
                ets[kc] = et
                if kc in extra:
                    extra[kc]()
            for kc in range(16 - D, 16):
                pv(kc)
            ht = None if h % 2 == 0 else rpool.tile([64, 1024], MM_DT, tag="ht")
            for qh in range(2):
                # 64-wide reciprocal of the replicated rowsum at partition
                # base 0 (custom-DVE ops only honor base 0 on HW), then one
                # multiply against ctx on partitions 64-127
                rc = rpool.tile([64, 512], F32, tag="rc")
                nc.vector.reciprocal_approx_fast(out=rc[:], in_=cps[qh][0:64, :])
                qcols = slice(qh * 512, (qh + 1) * 512)
                if h % 2 == 0:
                    nc.vector.tensor_mul(
                        ctxT[c][s1c][0:64, qcols], cps[qh][64:128, :], rc[:])
                else:
                    nc.vector.tensor_mul(
                        ht[:, qcols], cps[qh][64:128, :], rc[:])
            if h % 2 == 1:
                nc.sync.dma_start(out=ctxT[c][s1c][64:128, :], in_=ht[:])

        # s1c=0: V projections stream through head 0 (chunk j unlocks PV for
        # kc in [4j, 4j+4)); remaining DMAs (xv2/3, xq2/3, s1c=1 masks, woT)
        # are issued from inside the head stream so the serialized DMA device
        # delivers in need-order; Q[1024:2048] projections ride heads 2-3.
        head(0, 0, {3: lambda: proj_v(0),
                    7: lambda: proj_v(1),
                    11: lambda: proj_v(2),
                    15: lambda: proj_v(3)})
        head(0, 1, {1: lambda: load_x("q", 2),
                    5: lambda: load_x("q", 3)})
        head(0, 2, {7: lambda: proj_qk("q", Q4T, 2, "dve")})
        head(0, 3, {5: lambda: load_mask(1, 0),
                    7: lambda: proj_qk("q", Q4T, 3, "dve"),
                    9: lambda: load_mask(1, 1),
                    11: lambda: nc.sync.dma_start(
                        out=woT_sb[:],
                        in_=woT.ap().rearrange("(c p) n -> p c n", p=P)),
                    13: lambda: load_mask(1, 2)})
        # s1c=1: outproj for s1c=0 rides heads 0-1; s1c=1's own outproj
        # lands in the tail where ACT is free.
        head(1, 0, {1: lambda: load_mask(1, 3),
                    7: lambda: outproj(0, 0, "dve")})
        head(1, 1, {7: lambda: outproj(0, 1, "dve")})
        head(1, 3, {})
        head(1, 2, {})
        tailpools = [(pp_pj, "pj"), (pp_sc, "sc")]
        outproj(1, 0, "mix", pools=tailpools)
        outproj(1, 1, "mix", pools=tailpools)

    nc.compile()
    return nc


def get_nc():
    if "nc" not in _cache:
        _cache["nc"] = _build()
    return _cache["nc"]


def make_in_maps(q, k, v, mask, wQ_w, wQ_b, wK_w, wK_b, wV_w, wV_b, wO_w, wO_b):
    q = np.asarray(q, np.float32)
    k = np.asarray(k, np.float32)
    v = np.asarray(v, np.float32)
    mask = np.asarray(mask)

    def hilo(a):
        hi = a.astype(F8_NP)
        lo = (a - hi.astype(np.float32)).astype(F8_NP)
        return hi, lo

    xq_hi, xq_lo = hilo(np.ascontiguousarray(q.transpose(0, 2, 1)))
    xk_hi, xk_lo = hilo(np.ascontiguousarray(k.transpose(0, 2, 1)))
    xv_hi, xv_lo = hilo(np.ascontiguousarray(v.transpose(0, 2, 1)))
    mT = np.ascontiguousarray(mask[:, 0].transpose(0, 2, 1)).astype(MM_NP)
    in_maps = []
    for c in range(NCORES):
        b = c // GROUPS
        rows = slice((c % GROUPS) * HPC * DK, ((c % GROUPS) + 1) * HPC * DK)
        wqs = np.ascontiguousarray(np.asarray(wQ_w, np.float32)[rows].T) \
            * np.float32(SCALE * WS)
        wks = np.ascontiguousarray(np.asarray(wK_w, np.float32)[rows].T) \
            * np.float32(WS)
        wvs = np.ascontiguousarray(np.asarray(wV_w, np.float32)[rows].T) \
            * np.float32(WS)
        m = {
            "xq_hi": xq_hi[b], "xq_lo": xq_lo[b],
            "xk_hi": xk_hi[b], "xk_lo": xk_lo[b],
            "xv_hi": xv_hi[b], "xv_lo": xv_lo[b],
            "wqb": (np.asarray(wQ_b, np.float32)[rows] * np.float32(SCALE)).reshape(-1, 1),
            "wkb": np.asarray(wK_b, np.float32)[rows].reshape(-1, 1),
            "woT": np.ascontiguousarray(np.asarray(wO_w, np.float32)[:, rows].T).astype(MM_NP),
            "maskT": mT[b],
        }
        for nm, warr in (("q", wqs), ("k", wks), ("v", wvs)):
            hi, lo = hilo(warr)
            m[f"w{nm}_hi"] = hi
            m[f"w{nm}_lo"] = lo
        in_maps.append(m)
    return in_maps


def _get_runner():
    """Cached jitted 8-core runner (one XLA/walrus compile per process)."""
    if "runner" in _cache:
        return _cache["runner"]
    import jax
    from jax.sharding import Mesh, PartitionSpec, NamedSharding
    from jax.experimental.shard_map import shard_map
    from concourse.bass2jax import (
        _bass_exec_p, install_neuronx_cc_hook, partition_id_tensor)

    nc = get_nc()
    install_neuronx_cc_hook()
    pname = nc.partition_id_tensor.name if nc.partition_id_tensor else None
    in_names, out_names, out_avals = [], [], []
    for alloc in nc.m.functions[0].allocations:
        if not isinstance(alloc, mybir.MemoryLocationSet):
            continue
        name = alloc.memorylocations[0].name
        if alloc.kind == "ExternalInput":
            if name != pname:
                in_names.append(name)
        elif alloc.kind == "ExternalOutput":
            out_names.append(name)
            out_avals.append(jax.core.ShapedArray(
                tuple(alloc.tensor_shape), mybir.dt.np(alloc.dtype)))
    n_params = len(in_names)
    all_names = in_names + out_names
    if pname is not None:
        all_names = all_names + [pname]

    def _body(*args):
        operands = list(args)
        if pname is not None:
            operands.append(partition_id_tensor())
        outs = _bass_exec_p.bind(
            *operands,
            out_avals=tuple(out_avals),
            in_names=tuple(all_names),
            out_names=tuple(out_names),
            lowering_input_output_aliases=(),
            sim_require_finite=True,
            sim_require_nnan=True,
            nc=nc,
        )
        return tuple(outs)

    devices = jax.devices()[:NCORES]
    mesh = Mesh(np.asarray(devices), ("core",))
    nin = n_params + len(out_names)
    fn = jax.jit(shard_map(
        _body, mesh=mesh,
        in_specs=(PartitionSpec("core"),) * nin,
        out_specs=(PartitionSpec("core"),) * len(out_names),
        check_rep=False,
    ), keep_unused=True)
    sharding = NamedSharding(mesh, PartitionSpec("core"))
    zeros = [np.zeros((NCORES * a.shape[0], *a.shape[1:]), a.dtype)
             for a in out_avals]

    def run(in_maps):
        concat = [np.concatenate([np.asarray(m[n]) for m in in_maps], axis=0)
                  for n in in_names]
        args = [jax.device_put(x, sharding) for x in concat + zeros]
        outs = fn(*args)
        o = np.asarray(outs[0]).reshape(NCORES, S, DM)
        return [o[c] for c in range(NCORES)]

    _cache["runner"] = run
    return run


def kernel(q, k, v, mask, wQ_w, wQ_b, wK_w, wK_b, wV_w, wV_b, wO_w, wO_b):
    run = _get_runner()
    in_maps = make_in_maps(q, k, v, mask, wQ_w, wQ_b, wK_w, wK_b, wV_w, wV_b,
                           wO_w, wO_b)
    outs = run(in_maps)
    ob = (np.asarray(wO_b, np.float64)
          + np.asarray(wV_b, np.float64) @ np.asarray(wO_w, np.float64).T).astype(np.float32)
    full = np.empty((B, S, DM), np.float32)
    for b in range(B):
        acc = outs[b * GROUPS].astype(np.float32)
        for g in range(1, GROUPS):
            acc = acc + outs[b * GROUPS + g]
        full[b] = acc + ob[None, :]
    return full


# revision 53
# speedup vs baseline: 1.0206x; 1.0206x over previous
"""Multi-head attention (B=2, S=2048, D=1024, H=16) on 8 NeuronCores.

Sharding: core c handles batch b = c//4 and 4 heads starting at (c%4)*4
(data parallel over batch x tensor parallel over heads; wQ/wK/wV split
column-wise by head, wO row-wise; partial outputs summed on host).

v2: single fused pipeline, engines balanced against the cost model:
  - Projections in fp8 hi-lo DoubleRow form: x = x_hi + x_lo and w*64 =
    w_hi + w_lo (both fp8e4, host-prepared; the *64 keeps w out of fp8's
    subnormal range), accumulate x_hi*w_hi + x_hi*w_lo + x_lo*w_hi in psum
    (the dropped lo*lo term is ~2^-8 relative), scale 1/64 + bias applied at
    copyback. DoubleRow contracts 256 model-dims per matmul at 0.5 cyc/row,
    so a projection costs 0.75x its bf16 form in PE time at bf16-level
    accuracy.
  - Projections stream in 512-query-column chunks interleaved with the mask
    DMAs so the PE starts ~4us in: Q[0:1024] first (ACT copyback while ACT is
    idle), then K, V, Q[1024:2048] (DVE copybacks once the exp stream owns
    ACT).
  - Attention per (s1c query-1024-block, head): scoresT psum [128,1024]
    (2 bf16 matmuls), exp on ACT (psum -> sbuf bf16), {0,1}-mask multiply on
    DVE (bf16 2x mode), PV into two [128,512] psum chains. Each head's V4x
    block is [64 ones-columns | 64 V-columns], so the PV matmul lands the
    softmax denominator REPLICATED on psum partitions 0-63 and ctx on
    64-127 (out-free-size-only matmul cost makes the wide ones block free).
  - Normalization per chain: one 64-wide reciprocal_approx_fast at partition
    base 0 (custom-DVE ops only honor base 0 on real HW) straight from psum,
    then one multiply (mixed 32-aligned operand bases are legal). Odd heads
    restack to partitions 64-127 via a small batched SBUF-to-SBUF DMA.
  - PV for kc is emitted D kc-steps late so the PE queue never head-blocks
    on the exp chain of the same kc; the DMA issue order is hand-sequenced
    (K fully first: exp needs only scores, so the exp stream unblocks early
    while mask-muls and PV lag in the et pool until masks/V land).
  - Output projection per query half into [128,512] psums; copyback on DVE
    for s1c=0 (ACT is exp-saturated), alternating DVE/ACT in the tail for
    s1c=1; 256-row quarters ship as single batched DMAs. s1c=1 processes
    heads 0,1,3,2 so the final head needs no restack DMA on the tail path.
  - PSUM: scores 2x[128,1024] (4 banks) + PV 2x[128,512] (2) + proj/outproj
    2x[128,512] (2) = 8 banks; the tail outproj also borrows the idle
    scores pool for a 4-deep ring.

The wV bias is dropped on device: softmax rows sum to 1, so it contributes
exactly wV_b @ wO_w.T, folded into the host-side bias add. No max-subtraction
in softmax (scores are O(5), bf16 exp cannot overflow).

Host: out[b] = sum of the 4 cores' partials + (wO_b + wV_b @ wO_w.T).
"""

import numpy as np
from contextlib import ExitStack

import concourse.bacc as bacc
import concourse.tile as tile
from concourse import mybir
import ml_dtypes

B, S, DM, H, DK = 2, 2048, 1024, 16, 64
NCORES = 8
GROUPS = 4          # cores per batch
HPC = H // GROUPS   # heads per core = 4
P = 128
KT = DM // P        # 8 k-tiles over the model dim
CW = HPC * DK       # projected width per core = 256
SCALE = 1.0 / np.sqrt(DK)
WS = 64.0           # fp8 weight pre-scale (power of 2; undone at copyback)

MM_DT = mybir.dt.bfloat16
MM_NP = ml_dtypes.bfloat16
F8_DT = mybir.dt.float8e4
F8_NP = ml_dtypes.float8_e4m3

F32 = mybir.dt.float32
EXPF = mybir.ActivationFunctionType.Exp
IDENT = mybir.ActivationFunctionType.Identity
DR = mybir.MatmulPerfMode.DoubleRow

_cache: dict = {}


def _build():
    nc = bacc.Bacc("TRN2", target_bir_lowering=False, debug=False)

    xs = {}
    for nm in ("q", "k", "v"):
        for part in ("hi", "lo"):
            xs[(nm, part)] = nc.dram_tensor(f"x{nm}_{part}", [DM, S], F8_DT,
                                            kind="ExternalInput")
    ws = {}
    for nm in ("q", "k", "v"):
        for part in ("hi", "lo"):
            ws[(nm, part)] = nc.dram_tensor(f"w{nm}_{part}", [DM, CW], F8_DT,
                                            kind="ExternalInput")
    wqb = nc.dram_tensor("wqb", [CW, 1], F32, kind="ExternalInput")
    wkb = nc.dram_tensor("wkb", [CW, 1], F32, kind="ExternalInput")
    woT = nc.dram_tensor("woT", [CW, DM], MM_DT, kind="ExternalInput")
    maskT = nc.dram_tensor("maskT", [S, S], MM_DT, kind="ExternalInput")
    out = nc.dram_tensor("out", [S, DM], MM_DT, kind="ExternalOutput")

    with tile.TileContext(nc) as tc, ExitStack() as ctx:
        const = ctx.enter_context(tc.tile_pool(name="const", bufs=1))
        big = ctx.enter_context(tc.tile_pool(name="big", bufs=1))
        xpool = ctx.enter_context(tc.tile_pool(name="xpool", bufs=2))
        mpool = ctx.enter_context(tc.tile_pool(name="mpool", bufs=1))
        epool = ctx.enter_context(tc.tile_pool(name="epool", bufs=21))
        rpool = ctx.enter_context(tc.tile_pool(name="rpool", bufs=3))
        opool = ctx.enter_context(tc.tile_pool(name="opool", bufs=2))
        pp_sc = ctx.enter_context(tc.tile_pool(name="pp_sc", bufs=2, space="PSUM"))
        pp_ctx = ctx.enter_context(tc.tile_pool(name="pp_ctx", bufs=2, space="PSUM"))
        pp_pj = ctx.enter_context(tc.tile_pool(name="pp_pj", bufs=2, space="PSUM"))

        # persistent activations
        # Q4T/K4T: [dk-in-pair (part), pair c, s]; head h = c*2 + (p//64)
        Q4T = big.tile([P, 2, S], MM_DT, name="Q4T")
        K4T = big.tile([P, 2, S], MM_DT, name="K4T")
        V4x = big.tile([P, 16, HPC * 2 * DK], MM_DT, name="V4x")
        ctxT = [[big.tile([P, 1024], MM_DT, name=f"ctxT{i}_{j}") for j in range(2)]
                for i in range(2)]

        # 64 ones columns per head (cols h*128..h*128+63): the PV matmul then
        # lands the softmax denominator replicated on psum partitions 0-63
        # (reciprocal_approx_fast only honors partition base 0 on HW), and
        # ctx on partitions 64-127.
        for h in range(HPC):
            nc.vector.memset(V4x[:, :, h * 128 : h * 128 + 64], 1.0)

        w_sbs = {}
        b_sbs = {}

        def load_w(nm, bd):
            for part in ("hi", "lo"):
                w_sb = const.tile([P, KT, CW], F8_DT, name=f"w{nm}{part}_sb")
                nc.sync.dma_start(
                    out=w_sb[:],
                    in_=ws[(nm, part)].ap().rearrange("(t p) m -> p t m", p=P))
                w_sbs[(nm, part)] = w_sb
            if bd is not None:
                b_sb = const.tile([P, 2], F32, name=f"b{nm}_sb")
                nc.sync.dma_start(
                    out=b_sb[:], in_=bd.ap().rearrange("(m p) o -> p (m o)", p=P))
                b_sbs[nm] = b_sb

        x_tiles = {}
        def load_x(nm, sc, bufs=2):
            """Issue DMA for x columns [sc*512, (sc+1)*512] (hi+lo)."""
            ts = []
            for part in ("hi", "lo"):
                t = xpool.tile([P, KT, 512], F8_DT, tag=f"x{nm}{part}", bufs=bufs)
                nc.sync.dma_start(
                    out=t[:],
                    in_=xs[(nm, part)].ap().rearrange("(t p) s -> p t s", p=P)
                    [:, :, sc * 512:(sc + 1) * 512])
                ts.append(t)
            x_tiles[(nm, sc)] = ts

        def proj_qk(nm, dst, sc, eng):
            """Project x cols [sc*512,(sc+1)*512] -> dst[:, :, cols] (+bias)."""
            xh, xl = x_tiles.pop((nm, sc))
            wh, wl = w_sbs[(nm, "hi")], w_sbs[(nm, "lo")]
            b_sb = b_sbs[nm]
            for m in range(2):
                ps = pp_pj.tile([P, 512], F32, tag="pj", name="ps_pj")
                i = 0
                for wt, xt in ((wh, xh), (wh, xl), (wl, xh)):
                    for c in range(4):
                        nc.tensor.matmul(
                            ps[:], wt[:, 2 * c:2 * c + 2, m * 128:(m + 1) * 128],
                            xt[:, 2 * c:2 * c + 2, :],
                            start=(i == 0), stop=(i == 11), perf_mode=DR)
                        i += 1
                dcols = dst[:, m, sc * 512:(sc + 1) * 512]
                if eng == "act":
                    nc.scalar.activation(dcols, ps[:], IDENT,
                                         bias=b_sb[:, m:m + 1], scale=1.0 / WS)
                else:
                    nc.vector.tensor_scalar(
                        out=dcols, in0=ps[:], scalar1=1.0 / WS,
                        scalar2=b_sb[:, m:m + 1],
                        op0=mybir.AluOpType.mult, op1=mybir.AluOpType.add)

        def proj_v(sc):
            """Project v cols [sc*512,(sc+1)*512] (4 s-tiles of 128)."""
            xh, xl = x_tiles.pop(("v", sc))
            wh, wl = w_sbs[("v", "hi")], w_sbs[("v", "lo")]
            for st in range(4):
                ps = pp_pj.tile([P, 512], F32, tag="pj", name="ps_pj")
                i = 0
                for xt, wt in ((xh, wh), (xh, wl), (xl, wh)):
                    for c in range(4):
                        nc.tensor.matmul(
                            ps[:, 0:CW],
                            xt[:, 2 * c:2 * c + 2, st * 128:(st + 1) * 128],
                            wt[:, 2 * c:2 * c + 2, :],
                            start=(i == 0), stop=(i == 11), perf_mode=DR)
                        i += 1
                sidx = sc * 4 + st
                nc.vector.tensor_scalar(
                    out=V4x.rearrange("p s (h e) -> p s h e", e=2 * DK)[:, sidx, :, DK:2 * DK],
                    in0=ps[:, 0:CW].rearrange("p (h e) -> p h e", e=DK),
                    scalar1=1.0 / WS, scalar2=None, op0=mybir.AluOpType.mult)

        # mask chunks: [(s1c, mc)] covers kc in [mc*4, mc*4+4) for query block s1c
        mask_sb = {}
        def load_mask(s1c, mc):
            mt = mpool.tile([P, 4, 1024], MM_DT, tag=f"m{mc}")
            nc.sync.dma_start(
                out=mt[:],
                in_=maskT.ap().rearrange("(t p) s -> p t s", p=P)
                [:, mc * 4:(mc + 1) * 4, s1c * 1024:(s1c + 1) * 1024])
            mask_sb[(s1c, mc)] = mt

        # ---------------- lead-in: DMA issue order + Q/K projections --------
        # DMA device is serialized; this issue order is the delivery order.
        # K ships first and completely: exp needs only scores (K x Q), so the
        # whole exp stream unblocks early; mask-muls and PV lag behind in the
        # et pool until the masks/V land.
        load_w("k", wkb)
        load_x("k", 0, bufs=3)
        load_w("q", wqb)
        load_x("q", 0)
        load_x("q", 1)
        load_x("k", 1, bufs=3)
        load_x("k", 2, bufs=3)
        load_x("k", 3, bufs=3)
        load_w("v", None)
        load_mask(0, 0)
        load_x("v", 0)
        load_mask(0, 1)
        load_x("v", 1)
        load_mask(0, 2)
        load_x("v", 2)
        load_mask(0, 3)
        load_x("v", 3)
        woT_sb = const.tile([P, 2, DM], MM_DT, name="woT_sb")

        proj_qk("k", K4T, 0, "act")
        proj_qk("q", Q4T, 0, "act")
        proj_qk("q", Q4T, 1, "act")
        for sc in range(1, 4):
            proj_qk("k", K4T, sc, "dve")

        # ---------------- attention ----------------
        def outproj(s1c, qh, eng, pools=None, only_sb=None):
            """Output projection for query rows [s1c*1024 + qh*512 ...+512].

            Copybacks collect into [P, 2, 2, 512] staging tiles so each
            256-row quarter ships as a single DMA (HWDGE issue overhead
            dominates these short transfers). `pools` can alternate psum
            pools (the scores pool is idle in the tail)."""
            if pools is None:
                pools = [(pp_pj, "pj")]
            pi = 0
            for sb in ((0, 1) if only_sb is None else (only_sb,)):
                ob = opool.tile([P, 2, 2, 512], MM_DT, tag="ob")
                for st in range(2):
                    q0 = qh * 512 + (sb * 2 + st) * 128
                    for n2 in range(2):
                        pool, ptag = pools[pi % len(pools)]
                        pi += 1
                        ps = pool.tile([P, 512], F32, tag=ptag, name="ps_pj")
                        for c2 in range(2):
                            nc.tensor.matmul(
                                ps[:], ctxT[c2][s1c][:, q0:q0 + 128],
                                woT_sb[:, c2, n2 * 512:(n2 + 1) * 512],
                                start=(c2 == 0), stop=(c2 == 1))
                        e = eng if eng != "mix" else ("act" if (st + n2) % 2 else "dve")
                        if e == "act":
                            nc.scalar.activation(ob[:, st, n2, :], ps[:], IDENT)
                        else:
                            nc.vector.tensor_copy(out=ob[:, st, n2, :], in_=ps[:])
                row = s1c * 1024 + qh * 512 + sb * 256
                nc.sync.dma_start(
                    out=out.ap()[row:row + 256, :]
                    .rearrange("(t p) (n2 n) -> p t n2 n", p=P, n2=2),
                    in_=ob[:])

        def head(s1c, h, extra):
            """One head's scoresT/exp/mask/PV stream for query block s1c.

            `extra` maps kc -> thunk emitted after that kc's scores/exp/mask
            (projection or outproj work interleaved into this head's stream).
            PV for kc is emitted D steps late so the PE queue never
            head-blocks on the exp+mask chain of the same kc.
            """
            D = 5 if extra else 3
            base = (h % 2) * 64
            c = h // 2
            cps = [pp_ctx.tile([P, 512], F32, tag="cps", name=f"cps{i}")
                   for i in range(2)]

            def pv(kc):
                for qh in range(2):
                    nc.tensor.matmul(
                        cps[qh][:], V4x[:, kc, h * 128:(h + 1) * 128],
                        ets[kc][:, qh * 512:(qh + 1) * 512],
                        start=(kc == 0), stop=(kc == 15))

            ets = {}
            for kc in range(16):
                # PV first: it is ready (its et landed D tiles ago), so it
                # fills the PE queue while the scores matmul below waits for
                # its psum buffer to be drained by ACT.
                if kc >= D:
                    pv(kc - D)
                ps = pp_sc.tile([P, 1024], F32, tag="sc", name="ps_sc")
                lhsT = K4T[base:base + 64, c, kc * 128:(kc + 1) * 128]
                for n2 in range(2):
                    col = s1c * 1024 + n2 * 512
                    nc.tensor.matmul(
                        ps[:, n2 * 512:(n2 + 1) * 512], lhsT,
                        Q4T[base:base + 64, c, col:col + 512],
                        start=True, stop=True)
                et = epool.tile([P, 1024], MM_DT, tag="et")
                nc.scalar.activation(et[:], ps[:], EXPF)
                nc.vector.tensor_mul(
                    et[:], et[:], mask_sb[(s1c, kc // 4)][:, kc % 4, :])
                ets[kc] = et
                if kc in extra:
                    extra[kc]()
            for kc in range(16 - D, 16):
                pv(kc)
            ht = None if h % 2 == 0 else rpool.tile([64, 1024], MM_DT, tag="ht")
            for qh in range(2):
                # 64-wide reciprocal of the replicated rowsum at partition
                # base 0 (custom-DVE ops only honor base 0 on HW), then one
                # multiply against ctx on partitions 64-127
                rc = rpool.tile([64, 512], F32, tag="rc")
                nc.vector.reciprocal_approx_fast(out=rc[:], in_=cps[qh][0:64, :])
                qcols = slice(qh * 512, (qh + 1) * 512)
                if h % 2 == 0:
                    nc.vector.tensor_mul(
                        ctxT[c][s1c][0:64, qcols], cps[qh][64:128, :], rc[:])
                else:
                    nc.vector.tensor_mul(
                        ht[:, qcols], cps[qh][64:128, :], rc[:])
            if h % 2 == 1:
                nc.sync.dma_start(out=ctxT[c][s1c][64:128, :], in_=ht[:])

        # s1c=0: V projections stream through head 0 (chunk j unlocks PV for
        # kc in [4j, 4j+4)); remaining DMAs (xv2/3, xq2/3, s1c=1 masks, woT)
        # are issued from inside the head stream so the serialized DMA device
        # delivers in need-order; Q[1024:2048] projections ride heads 2-3.
        head(0, 0, {3: lambda: proj_v(0),
                    7: lambda: proj_v(1),
                    11: lambda: proj_v(2),
                    15: lambda: proj_v(3)})
        head(0, 1, {1: lambda: load_x("q", 2),
                    5: lambda: load_x("q", 3)})
        head(0, 2, {7: lambda: proj_qk("q", Q4T, 2, "dve")})
        head(0, 3, {5: lambda: load_mask(1, 0),
                    7: lambda: proj_qk("q", Q4T, 3, "dve"),
                    9: lambda: load_mask(1, 1),
                    11: lambda: nc.sync.dma_start(
                        out=woT_sb[:],
                        in_=woT.ap().rearrange("(c p) n -> p c n", p=P)),
                    13: lambda: load_mask(1, 2)})
        # s1c=1: outproj for s1c=0 rides heads 0-1; s1c=1's own outproj
        # lands in the tail where ACT is free.
        head(1, 0, {1: lambda: load_mask(1, 3),
                    5: lambda: outproj(0, 0, "dve", only_sb=0)})
        head(1, 1, {5: lambda: outproj(0, 0, "dve", only_sb=1)})
        head(1, 3, {5: lambda: outproj(0, 1, "dve", only_sb=0)})
        head(1, 2, {5: lambda: outproj(0, 1, "dve", only_sb=1)})
        tailpools = [(pp_pj, "pj"), (pp_sc, "sc")]
        outproj(1, 0, "mix", pools=tailpools)
        outproj(1, 1, "mix", pools=tailpools)

    nc.compile()
    return nc


def get_nc():
    if "nc" not in _cache:
        _cache["nc"] = _build()
    return _cache["nc"]


def make_in_maps(q, k, v, mask, wQ_w, wQ_b, wK_w, wK_b, wV_w, wV_b, wO_w, wO_b):
    q = np.asarray(q, np.float32)
    k = np.asarray(k, np.float32)
    v = np.asarray(v, np.float32)
    mask = np.asarray(mask)

    def hilo(a):
        hi = a.astype(F8_NP)
        lo = (a - hi.astype(np.float32)).astype(F8_NP)
        return hi, lo

    xq_hi, xq_lo = hilo(np.ascontiguousarray(q.transpose(0, 2, 1)))
    xk_hi, xk_lo = hilo(np.ascontiguousarray(k.transpose(0, 2, 1)))
    xv_hi, xv_lo = hilo(np.ascontiguousarray(v.transpose(0, 2, 1)))
    mT = np.ascontiguousarray(mask[:, 0].transpose(0, 2, 1)).astype(MM_NP)
    in_maps = []
    for c in range(NCORES):
        b = c // GROUPS
        rows = slice((c % GROUPS) * HPC * DK, ((c % GROUPS) + 1) * HPC * DK)
        wqs = np.ascontiguousarray(np.asarray(wQ_w, np.float32)[rows].T) \
            * np.float32(SCALE * WS)
        wks = np.ascontiguousarray(np.asarray(wK_w, np.float32)[rows].T) \
            * np.float32(WS)
        wvs = np.ascontiguousarray(np.asarray(wV_w, np.float32)[rows].T) \
            * np.float32(WS)
        m = {
            "xq_hi": xq_hi[b], "xq_lo": xq_lo[b],
            "xk_hi": xk_hi[b], "xk_lo": xk_lo[b],
            "xv_hi": xv_hi[b], "xv_lo": xv_lo[b],
            "wqb": (np.asarray(wQ_b, np.float32)[rows] * np.float32(SCALE)).reshape(-1, 1),
            "wkb": np.asarray(wK_b, np.float32)[rows].reshape(-1, 1),
            "woT": np.ascontiguousarray(np.asarray(wO_w, np.float32)[:, rows].T).astype(MM_NP),
            "maskT": mT[b],
        }
        for nm, warr in (("q", wqs), ("k", wks), ("v", wvs)):
            hi, lo = hilo(warr)
            m[f"w{nm}_hi"] = hi
            m[f"w{nm}_lo"] = lo
        in_maps.append(m)
    return in_maps


def _get_runner():
    """Cached jitted 8-core runner (one XLA/walrus compile per process)."""
    if "runner" in _cache:
        return _cache["runner"]
    import jax
    from jax.sharding import Mesh, PartitionSpec, NamedSharding
    from jax.experimental.shard_map import shard_map
    from concourse.bass2jax import (
        _bass_exec_p, install_neuronx_cc_hook, partition_id_tensor)

    nc = get_nc()
    install_neuronx_cc_hook()
    pname = nc.partition_id_tensor.name if nc.partition_id_tensor else None
    in_names, out_names, out_avals = [], [], []
    for alloc in nc.m.functions[0].allocations:
        if not isinstance(alloc, mybir.MemoryLocationSet):
            continue
        name = alloc.memorylocations[0].name
        if alloc.kind == "ExternalInput":
            if name != pname:
                in_names.append(name)
        elif alloc.kind == "ExternalOutput":
            out_names.append(name)
            out_avals.append(jax.core.ShapedArray(
                tuple(alloc.tensor_shape), mybir.dt.np(alloc.dtype)))
    n_params = len(in_names)
    all_names = in_names + out_names
    if pname is not None:
        all_names = all_names + [pname]

    def _body(*args):
        operands = list(args)
        if pname is not None:
            operands.append(partition_id_tensor())
        outs = _bass_exec_p.bind(
            *operands,
            out_avals=tuple(out_avals),
            in_names=tuple(all_names),
            out_names=tuple(out_names),
            lowering_input_output_aliases=(),
            sim_require_finite=True,
            sim_require_nnan=True,
            nc=nc,
        )
        return tuple(outs)

    devices = jax.devices()[:NCORES]
    mesh = Mesh(np.asarray(devices), ("core",))
    nin = n_params + len(out_names)
    fn = jax.jit(shard_map(
        _body, mesh=mesh,
        in_specs=(PartitionSpec("core"),) * nin,
        out_specs=(PartitionSpec("core"),) * len(out_names),
        check_rep=False,
    ), keep_unused=True)
    sharding = NamedSharding(mesh, PartitionSpec("core"))
    zeros = [np.zeros((NCORES * a.shape[0], *a.shape[1:]), a.dtype)
             for a in out_avals]

    def run(in_maps):
        concat = [np.concatenate([np.asarray(m[n]) for m in in_maps], axis=0)
                  for n in in_names]
        args = [jax.device_put(x, sharding) for x in concat + zeros]
        outs = fn(*args)
        o = np.asarray(outs[0]).reshape(NCORES, S, DM)
        return [o[c] for c in range(NCORES)]

    _cache["runner"] = run
    return run


def kernel(q, k, v, mask, wQ_w, wQ_b, wK_w, wK_b, wV_w, wV_b, wO_w, wO_b):
    run = _get_runner()
    in_maps = make_in_maps(q, k, v, mask, wQ_w, wQ_b, wK_w, wK_b, wV_w, wV_b,
                           wO_w, wO_b)
    outs = run(in_maps)
    ob = (np.asarray(wO_b, np.float64)
          + np.asarray(wV_b, np.float64) @ np.asarray(wO_w, np.float64).T).astype(np.float32)
    full = np.empty((B, S, DM), np.float32)
    for b in range(B):
        acc = outs[b * GROUPS].astype(np.float32)
        for g in range(1, GROUPS):
            acc = acc + outs[b * GROUPS + g]
        full[b] = acc + ob[None, :]
    return full


# revision 56
# speedup vs baseline: 1.0216x; 1.0010x over previous
"""Multi-head attention (B=2, S=2048, D=1024, H=16) on 8 NeuronCores.

Sharding: core c handles batch b = c//4 and 4 heads starting at (c%4)*4
(data parallel over batch x tensor parallel over heads; wQ/wK/wV split
column-wise by head, wO row-wise; partial outputs summed on host).

v2: single fused pipeline, engines balanced against the cost model:
  - Projections in fp8 hi-lo DoubleRow form: x = x_hi + x_lo and w*64 =
    w_hi + w_lo (both fp8e4, host-prepared; the *64 keeps w out of fp8's
    subnormal range), accumulate x_hi*w_hi + x_hi*w_lo + x_lo*w_hi in psum
    (the dropped lo*lo term is ~2^-8 relative), scale 1/64 + bias applied at
    copyback. DoubleRow contracts 256 model-dims per matmul at 0.5 cyc/row,
    so a projection costs 0.75x its bf16 form in PE time at bf16-level
    accuracy.
  - Projections stream in 512-query-column chunks interleaved with the mask
    DMAs so the PE starts ~4us in: Q[0:1024] first (ACT copyback while ACT is
    idle), then K, V, Q[1024:2048] (DVE copybacks once the exp stream owns
    ACT).
  - Attention per (s1c query-1024-block, head): scoresT psum [128,1024]
    (2 bf16 matmuls), exp on ACT (psum -> sbuf bf16), {0,1}-mask multiply on
    DVE (bf16 2x mode), PV into two [128,512] psum chains. Each head's V4x
    block is [64 ones-columns | 64 V-columns], so the PV matmul lands the
    softmax denominator REPLICATED on psum partitions 0-63 and ctx on
    64-127 (out-free-size-only matmul cost makes the wide ones block free).
  - Normalization per chain: one 64-wide reciprocal_approx_fast at partition
    base 0 (custom-DVE ops only honor base 0 on real HW) straight from psum,
    then one multiply (mixed 32-aligned operand bases are legal). Odd heads
    restack to partitions 64-127 via a small batched SBUF-to-SBUF DMA.
  - PV for kc is emitted D kc-steps late so the PE queue never head-blocks
    on the exp chain of the same kc; the DMA issue order is hand-sequenced
    (K fully first: exp needs only scores, so the exp stream unblocks early
    while mask-muls and PV lag in the et pool until masks/V land).
  - Output projection per query half into [128,512] psums; copyback on DVE
    for s1c=0 (ACT is exp-saturated), alternating DVE/ACT in the tail for
    s1c=1; 256-row quarters ship as single batched DMAs. s1c=1 processes
    heads 0,1,3,2 so the final head needs no restack DMA on the tail path.
  - PSUM: scores 2x[128,1024] (4 banks) + PV 2x[128,512] (2) + proj/outproj
    2x[128,512] (2) = 8 banks; the tail outproj also borrows the idle
    scores pool for a 4-deep ring.

The wV bias is dropped on device: softmax rows sum to 1, so it contributes
exactly wV_b @ wO_w.T, folded into the host-side bias add. No max-subtraction
in softmax (scores are O(5), bf16 exp cannot overflow).

Host: out[b] = sum of the 4 cores' partials + (wO_b + wV_b @ wO_w.T).
"""

import numpy as np
from contextlib import ExitStack

import concourse.bacc as bacc
import concourse.tile as tile
from concourse import mybir
import ml_dtypes

B, S, DM, H, DK = 2, 2048, 1024, 16, 64
NCORES = 8
GROUPS = 4          # cores per batch
HPC = H // GROUPS   # heads per core = 4
P = 128
KT = DM // P        # 8 k-tiles over the model dim
CW = HPC * DK       # projected width per core = 256
SCALE = 1.0 / np.sqrt(DK)
WS = 64.0           # fp8 weight pre-scale (power of 2; undone at copyback)

MM_DT = mybir.dt.bfloat16
MM_NP = ml_dtypes.bfloat16
F8_DT = mybir.dt.float8e4
F8_NP = ml_dtypes.float8_e4m3

F32 = mybir.dt.float32
EXPF = mybir.ActivationFunctionType.Exp
IDENT = mybir.ActivationFunctionType.Identity
DR = mybir.MatmulPerfMode.DoubleRow

_cache: dict = {}


def _build():
    nc = bacc.Bacc("TRN2", target_bir_lowering=False, debug=False)

    xs = {}
    for nm in ("q", "k", "v"):
        for part in ("hi", "lo"):
            xs[(nm, part)] = nc.dram_tensor(f"x{nm}_{part}", [DM, S], F8_DT,
                                            kind="ExternalInput")
    ws = {}
    for nm in ("q", "k", "v"):
        for part in ("hi", "lo"):
            ws[(nm, part)] = nc.dram_tensor(f"w{nm}_{part}", [DM, CW], F8_DT,
                                            kind="ExternalInput")
    wqb = nc.dram_tensor("wqb", [CW, 1], F32, kind="ExternalInput")
    wkb = nc.dram_tensor("wkb", [CW, 1], F32, kind="ExternalInput")
    woT = nc.dram_tensor("woT", [CW, DM], MM_DT, kind="ExternalInput")
    maskT = nc.dram_tensor("maskT", [S, S], MM_DT, kind="ExternalInput")
    out = nc.dram_tensor("out", [S, DM], MM_DT, kind="ExternalOutput")

    with tile.TileContext(nc) as tc, ExitStack() as ctx:
        const = ctx.enter_context(tc.tile_pool(name="const", bufs=1))
        big = ctx.enter_context(tc.tile_pool(name="big", bufs=1))
        xpool = ctx.enter_context(tc.tile_pool(name="xpool", bufs=2))
        mpool = ctx.enter_context(tc.tile_pool(name="mpool", bufs=1))
        epool = ctx.enter_context(tc.tile_pool(name="epool", bufs=21))
        rpool = ctx.enter_context(tc.tile_pool(name="rpool", bufs=3))
        opool = ctx.enter_context(tc.tile_pool(name="opool", bufs=2))
        pp_sc = ctx.enter_context(tc.tile_pool(name="pp_sc", bufs=2, space="PSUM"))
        pp_ctx = ctx.enter_context(tc.tile_pool(name="pp_ctx", bufs=2, space="PSUM"))
        pp_pj = ctx.enter_context(tc.tile_pool(name="pp_pj", bufs=2, space="PSUM"))

        # persistent activations
        # Q4T/K4T: [dk-in-pair (part), pair c, s]; head h = c*2 + (p//64)
        Q4T = big.tile([P, 2, S], MM_DT, name="Q4T")
        K4T = big.tile([P, 2, S], MM_DT, name="K4T")
        V4x = big.tile([P, 16, HPC * 2 * DK], MM_DT, name="V4x")
        ctxT = [[big.tile([P, 1024], MM_DT, name=f"ctxT{i}_{j}") for j in range(2)]
                for i in range(2)]

        # 64 ones columns per head (cols h*128..h*128+63): the PV matmul then
        # lands the softmax denominator replicated on psum partitions 0-63
        # (reciprocal_approx_fast only honors partition base 0 on HW), and
        # ctx on partitions 64-127.
        for h in range(HPC):
            nc.vector.memset(V4x[:, :, h * 128 : h * 128 + 64], 1.0)

        w_sbs = {}
        b_sbs = {}

        def load_w_part(nm, part):
            w_sb = const.tile([P, KT, CW], F8_DT, name=f"w{nm}{part}_sb")
            nc.sync.dma_start(
                out=w_sb[:],
                in_=ws[(nm, part)].ap().rearrange("(t p) m -> p t m", p=P))
            w_sbs[(nm, part)] = w_sb

        def load_w(nm, bd):
            for part in ("hi", "lo"):
                load_w_part(nm, part)
            if bd is not None:
                b_sb = const.tile([P, 2], F32, name=f"b{nm}_sb")
                nc.sync.dma_start(
                    out=b_sb[:], in_=bd.ap().rearrange("(m p) o -> p (m o)", p=P))
                b_sbs[nm] = b_sb

        def load_bias(nm, bd):
            b_sb = const.tile([P, 2], F32, name=f"b{nm}_sb")
            nc.sync.dma_start(
                out=b_sb[:], in_=bd.ap().rearrange("(m p) o -> p (m o)", p=P))
            b_sbs[nm] = b_sb

        x_tiles = {}
        def load_x_part(nm, sc, part, bufs=2):
            t = xpool.tile([P, KT, 512], F8_DT, tag=f"x{nm}{part}", bufs=bufs)
            nc.sync.dma_start(
                out=t[:],
                in_=xs[(nm, part)].ap().rearrange("(t p) s -> p t s", p=P)
                [:, :, sc * 512:(sc + 1) * 512])
            x_tiles.setdefault((nm, sc), [None, None])[0 if part == "hi" else 1] = t

        def load_x(nm, sc, bufs=2):
            """Issue DMA for x columns [sc*512, (sc+1)*512] (hi+lo)."""
            for part in ("hi", "lo"):
                load_x_part(nm, sc, part, bufs=bufs)

        def proj_qk(nm, dst, sc, eng):
            """Project x cols [sc*512,(sc+1)*512] -> dst[:, :, cols] (+bias)."""
            xh, xl = x_tiles.pop((nm, sc))
            wh, wl = w_sbs[(nm, "hi")], w_sbs[(nm, "lo")]
            b_sb = b_sbs[nm]
            for m in range(2):
                ps = pp_pj.tile([P, 512], F32, tag="pj", name="ps_pj")
                i = 0
                for wt, xt in ((wh, xh), (wh, xl), (wl, xh)):
                    for c in range(4):
                        nc.tensor.matmul(
                            ps[:], wt[:, 2 * c:2 * c + 2, m * 128:(m + 1) * 128],
                            xt[:, 2 * c:2 * c + 2, :],
                            start=(i == 0), stop=(i == 11), perf_mode=DR)
                        i += 1
                dcols = dst[:, m, sc * 512:(sc + 1) * 512]
                if eng == "act":
                    nc.scalar.activation(dcols, ps[:], IDENT,
                                         bias=b_sb[:, m:m + 1], scale=1.0 / WS)
                else:
                    nc.vector.tensor_scalar(
                        out=dcols, in0=ps[:], scalar1=1.0 / WS,
                        scalar2=b_sb[:, m:m + 1],
                        op0=mybir.AluOpType.mult, op1=mybir.AluOpType.add)

        def proj_v(sc):
            """Project v cols [sc*512,(sc+1)*512] (4 s-tiles of 128)."""
            xh, xl = x_tiles.pop(("v", sc))
            wh, wl = w_sbs[("v", "hi")], w_sbs[("v", "lo")]
            for st in range(4):
                ps = pp_pj.tile([P, 512], F32, tag="pj", name="ps_pj")
                i = 0
                for xt, wt in ((xh, wh), (xh, wl), (xl, wh)):
                    for c in range(4):
                        nc.tensor.matmul(
                            ps[:, 0:CW],
                            xt[:, 2 * c:2 * c + 2, st * 128:(st + 1) * 128],
                            wt[:, 2 * c:2 * c + 2, :],
                            start=(i == 0), stop=(i == 11), perf_mode=DR)
                        i += 1
                sidx = sc * 4 + st
                nc.vector.tensor_scalar(
                    out=V4x.rearrange("p s (h e) -> p s h e", e=2 * DK)[:, sidx, :, DK:2 * DK],
                    in0=ps[:, 0:CW].rearrange("p (h e) -> p h e", e=DK),
                    scalar1=1.0 / WS, scalar2=None, op0=mybir.AluOpType.mult)

        # mask chunks: [(s1c, mc)] covers kc in [mc*4, mc*4+4) for query block s1c
        mask_sb = {}
        def load_mask(s1c, mc):
            mt = mpool.tile([P, 4, 1024], MM_DT, tag=f"m{mc}")
            nc.sync.dma_start(
                out=mt[:],
                in_=maskT.ap().rearrange("(t p) s -> p t s", p=P)
                [:, mc * 4:(mc + 1) * 4, s1c * 1024:(s1c + 1) * 1024])
            mask_sb[(s1c, mc)] = mt

        # ---------------- lead-in: DMA issue order + Q/K projections --------
        # DMA device is serialized; this issue order is the delivery order.
        # K ships first and completely: exp needs only scores (K x Q), so the
        # whole exp stream unblocks early; mask-muls and PV lag behind in the
        # et pool until the masks/V land.
        load_w_part("k", "hi")
        load_x_part("k", 0, "hi", bufs=3)
        load_x_part("k", 0, "lo", bufs=3)
        load_w_part("k", "lo")
        load_bias("k", wkb)
        load_w_part("q", "hi")
        load_x_part("q", 0, "hi")
        load_x_part("q", 0, "lo")
        load_w_part("q", "lo")
        load_bias("q", wqb)
        load_x("q", 1)
        load_x("k", 1, bufs=3)
        load_x("k", 2, bufs=3)
        load_x("k", 3, bufs=3)
        load_w("v", None)
        load_mask(0, 0)
        load_x("v", 0)
        load_mask(0, 1)
        load_x("v", 1)
        load_mask(0, 2)
        load_x("v", 2)
        load_mask(0, 3)
        load_x("v", 3)
        woT_sb = const.tile([P, 2, DM], MM_DT, name="woT_sb")

        proj_qk("k", K4T, 0, "act")
        proj_qk("q", Q4T, 0, "act")
        proj_qk("q", Q4T, 1, "act")
        for sc in range(1, 4):
            proj_qk("k", K4T, sc, "dve")

        # ---------------- attention ----------------
        def outproj(s1c, qh, eng, pools=None, only_sb=None):
            """Output projection for query rows [s1c*1024 + qh*512 ...+512].

            Copybacks collect into [P, 2, 2, 512] staging tiles so each
            256-row quarter ships as a single DMA (HWDGE issue overhead
            dominates these short transfers). `pools` can alternate psum
            pools (the scores pool is idle in the tail)."""
            if pools is None:
                pools = [(pp_pj, "pj")]
            pi = 0
            for sb in ((0, 1) if only_sb is None else (only_sb,)):
                ob = opool.tile([P, 2, 2, 512], MM_DT, tag="ob")
                for st in range(2):
                    q0 = qh * 512 + (sb * 2 + st) * 128
                    for n2 in range(2):
                        pool, ptag = pools[pi % len(pools)]
                        pi += 1
                        ps = pool.tile([P, 512], F32, tag=ptag, name="ps_pj")
                        for c2 in range(2):
                            nc.tensor.matmul(
                                ps[:], ctxT[c2][s1c][:, q0:q0 + 128],
                                woT_sb[:, c2, n2 * 512:(n2 + 1) * 512],
                                start=(c2 == 0), stop=(c2 == 1))
                        e = eng if eng != "mix" else ("act" if (st + n2) % 2 else "dve")
                        if e == "act":
                            nc.scalar.activation(ob[:, st, n2, :], ps[:], IDENT)
                        else:
                            nc.vector.tensor_copy(out=ob[:, st, n2, :], in_=ps[:])
                row = s1c * 1024 + qh * 512 + sb * 256
                nc.sync.dma_start(
                    out=out.ap()[row:row + 256, :]
                    .rearrange("(t p) (n2 n) -> p t n2 n", p=P, n2=2),
                    in_=ob[:])

        def head(s1c, h, extra):
            """One head's scoresT/exp/mask/PV stream for query block s1c.

            `extra` maps kc -> thunk emitted after that kc's scores/exp/mask
            (projection or outproj work interleaved into this head's stream).
            PV for kc is emitted D steps late so the PE queue never
            head-blocks on the exp+mask chain of the same kc.
            """
            D = 5 if extra else 3
            base = (h % 2) * 64
            c = h // 2
            cps = [pp_ctx.tile([P, 512], F32, tag="cps", name=f"cps{i}")
                   for i in range(2)]

            def pv(kc):
                for qh in range(2):
                    nc.tensor.matmul(
                        cps[qh][:], V4x[:, kc, h * 128:(h + 1) * 128],
                        ets[kc][:, qh * 512:(qh + 1) * 512],
                        start=(kc == 0), stop=(kc == 15))

            ets = {}
            for kc in range(16):
                # PV first: it is ready (its et landed D tiles ago), so it
                # fills the PE queue while the scores matmul below waits for
                # its psum buffer to be drained by ACT.
                if kc >= D:
                    pv(kc - D)
                ps = pp_sc.tile([P, 1024], F32, tag="sc", name="ps_sc")
                lhsT = K4T[base:base + 64, c, kc * 128:(kc + 1) * 128]
                for n2 in range(2):
                    col = s1c * 1024 + n2 * 512
                    nc.tensor.matmul(
                        ps[:, n2 * 512:(n2 + 1) * 512], lhsT,
                        Q4T[base:base + 64, c, col:col + 512],
                        start=True, stop=True)
                et = epool.tile([P, 1024], MM_DT, tag="et")
                nc.scalar.activation(et[:], ps[:], EXPF)
                nc.vector.tensor_mul(
                    et[:], et[:], mask_sb[(s1c, kc // 4)][:, kc % 4, :])
                ets[kc] = et
                if kc in extra:
                    extra[kc]()
            for kc in range(16 - D, 16):
                pv(kc)
            ht = None if h % 2 == 0 else rpool.tile([64, 1024], MM_DT, tag="ht")
            for qh in range(2):
                # 64-wide reciprocal of the replicated rowsum at partition
                # base 0 (custom-DVE ops only honor base 0 on HW), then one
                # multiply against ctx on partitions 64-127
                rc = rpool.tile([64, 512], F32, tag="rc")
                nc.vector.reciprocal_approx_fast(out=rc[:], in_=cps[qh][0:64, :])
                qcols = slice(qh * 512, (qh + 1) * 512)
                if h % 2 == 0:
                    nc.vector.tensor_mul(
                        ctxT[c][s1c][0:64, qcols], cps[qh][64:128, :], rc[:])
                else:
                    nc.vector.tensor_mul(
                        ht[:, qcols], cps[qh][64:128, :], rc[:])
            if h % 2 == 1:
                nc.sync.dma_start(out=ctxT[c][s1c][64:128, :], in_=ht[:])

        # s1c=0: V projections stream through head 0 (chunk j unlocks PV for
        # kc in [4j, 4j+4)); remaining DMAs (xv2/3, xq2/3, s1c=1 masks, woT)
        # are issued from inside the head stream so the serialized DMA device
        # delivers in need-order; Q[1024:2048] projections ride heads 2-3.
        head(0, 0, {3: lambda: proj_v(0),
                    7: lambda: proj_v(1),
                    11: lambda: proj_v(2),
                    15: lambda: proj_v(3)})
        head(0, 1, {1: lambda: load_x("q", 2),
                    5: lambda: load_x("q", 3)})
        head(0, 2, {7: lambda: proj_qk("q", Q4T, 2, "dve")})
        head(0, 3, {5: lambda: load_mask(1, 0),
                    7: lambda: proj_qk("q", Q4T, 3, "dve"),
                    9: lambda: load_mask(1, 1),
                    11: lambda: nc.sync.dma_start(
                        out=woT_sb[:],
                        in_=woT.ap().rearrange("(c p) n -> p c n", p=P)),
                    13: lambda: load_mask(1, 2)})
        # s1c=1: outproj for s1c=0 rides heads 0-1; s1c=1's own outproj
        # lands in the tail where ACT is free.
        head(1, 0, {1: lambda: load_mask(1, 3),
                    5: lambda: outproj(0, 0, "dve", only_sb=0)})
        head(1, 1, {5: lambda: outproj(0, 0, "dve", only_sb=1)})
        head(1, 3, {5: lambda: outproj(0, 1, "dve", only_sb=0)})
        head(1, 2, {5: lambda: outproj(0, 1, "dve", only_sb=1)})
        tailpools = [(pp_pj, "pj"), (pp_sc, "sc")]
        outproj(1, 0, "mix", pools=tailpools)
        outproj(1, 1, "mix", pools=tailpools)

    nc.compile()
    return nc


def get_nc():
    if "nc" not in _cache:
        _cache["nc"] = _build()
    return _cache["nc"]


def make_in_maps(q, k, v, mask, wQ_w, wQ_b, wK_w, wK_b, wV_w, wV_b, wO_w, wO_b):
    q = np.asarray(q, np.float32)
    k = np.asarray(k, np.float32)
    v = np.asarray(v, np.float32)
    mask = np.asarray(mask)

    def hilo(a):
        hi = a.astype(F8_NP)
        lo = (a - hi.astype(np.float32)).astype(F8_NP)
        return hi, lo

    xq_hi, xq_lo = hilo(np.ascontiguousarray(q.transpose(0, 2, 1)))
    xk_hi, xk_lo = hilo(np.ascontiguousarray(k.transpose(0, 2, 1)))
    xv_hi, xv_lo = hilo(np.ascontiguousarray(v.transpose(0, 2, 1)))
    mT = np.ascontiguousarray(mask[:, 0].transpose(0, 2, 1)).astype(MM_NP)
    in_maps = []
    for c in range(NCORES):
        b = c // GROUPS
        rows = slice((c % GROUPS) * HPC * DK, ((c % GROUPS) + 1) * HPC * DK)
        wqs = np.ascontiguousarray(np.asarray(wQ_w, np.float32)[rows].T) \
            * np.float32(SCALE * WS)
        wks = np.ascontiguousarray(np.asarray(wK_w, np.float32)[rows].T) \
            * np.float32(WS)
        wvs = np.ascontiguousarray(np.asarray(wV_w, np.float32)[rows].T) \
            * np.float32(WS)
        m = {
            "xq_hi": xq_hi[b], "xq_lo": xq_lo[b],
            "xk_hi": xk_hi[b], "xk_lo": xk_lo[b],
            "xv_hi": xv_hi[b], "xv_lo": xv_lo[b],
            "wqb": (np.asarray(wQ_b, np.float32)[rows] * np.float32(SCALE)).reshape(-1, 1),
            "wkb": np.asarray(wK_b, np.float32)[rows].reshape(-1, 1),
            "woT": np.ascontiguousarray(np.asarray(wO_w, np.float32)[:, rows].T).astype(MM_NP),
            "maskT": mT[b],
        }
        for nm, warr in (("q", wqs), ("k", wks), ("v", wvs)):
            hi, lo = hilo(warr)
            m[f"w{nm}_hi"] = hi
            m[f"w{nm}_lo"] = lo
        in_maps.append(m)
    return in_maps


def _get_runner():
    """Cached jitted 8-core runner (one XLA/walrus compile per process)."""
    if "runner" in _cache:
        return _cache["runner"]
    import jax
    from jax.sharding import Mesh, PartitionSpec, NamedSharding
    from jax.experimental.shard_map import shard_map
    from concourse.bass2jax import (
        _bass_exec_p, install_neuronx_cc_hook, partition_id_tensor)

    nc = get_nc()
    install_neuronx_cc_hook()
    pname = nc.partition_id_tensor.name if nc.partition_id_tensor else None
    in_names, out_names, out_avals = [], [], []
    for alloc in nc.m.functions[0].allocations:
        if not isinstance(alloc, mybir.MemoryLocationSet):
            continue
        name = alloc.memorylocations[0].name
        if alloc.kind == "ExternalInput":
            if name != pname:
                in_names.append(name)
        elif alloc.kind == "ExternalOutput":
            out_names.append(name)
            out_avals.append(jax.core.ShapedArray(
                tuple(alloc.tensor_shape), mybir.dt.np(alloc.dtype)))
    n_params = len(in_names)
    all_names = in_names + out_names
    if pname is not None:
        all_names = all_names + [pname]

    def _body(*args):
        operands = list(args)
        if pname is not None:
            operands.append(partition_id_tensor())
        outs = _bass_exec_p.bind(
            *operands,
            out_avals=tuple(out_avals),
            in_names=tuple(all_names),
            out_names=tuple(out_names),
            lowering_input_output_aliases=(),
            sim_require_finite=True,
            sim_require_nnan=True,
            nc=nc,
        )
        return tuple(outs)

    devices = jax.devices()[:NCORES]
    mesh = Mesh(np.asarray(devices), ("core",))
    nin = n_params + len(out_names)
    fn = jax.jit(shard_map(
        _body, mesh=mesh,
        in_specs=(PartitionSpec("core"),) * nin,
        out_specs=(PartitionSpec("core"),) * len(out_names),
        check_rep=False,
    ), keep_unused=True)
    sharding = NamedSharding(mesh, PartitionSpec("core"))
    zeros = [np.zeros((NCORES * a.shape[0], *a.shape[1:]), a.dtype)
             for a in out_avals]

    def run(in_maps):
        concat = [np.concatenate([np.asarray(m[n]) for m in in_maps], axis=0)
                  for n in in_names]
        args = [jax.device_put(x, sharding) for x in concat + zeros]
        outs = fn(*args)
        o = np.asarray(outs[0]).reshape(NCORES, S, DM)
        return [o[c] for c in range(NCORES)]

    _cache["runner"] = run
    return run


def kernel(q, k, v, mask, wQ_w, wQ_b, wK_w, wK_b, wV_w, wV_b, wO_w, wO_b):
    run = _get_runner()
    in_maps = make_in_maps(q, k, v, mask, wQ_w, wQ_b, wK_w, wK_b, wV_w, wV_b,
                           wO_w, wO_b)
    outs = run(in_maps)
    ob = (np.asarray(wO_b, np.float64)
          + np.asarray(wV_b, np.float64) @ np.asarray(wO_w, np.float64).T).astype(np.float32)
    full = np.empty((B, S, DM), np.float32)
    for b in range(B):
        acc = outs[b * GROUPS].astype(np.float32)
        for g in range(1, GROUPS):
            acc = acc + outs[b * GROUPS + g]
        full[b] = acc + ob[None, :]
    return full


# revision 61
# speedup vs baseline: 1.0317x; 1.0099x over previous
"""Multi-head attention (B=2, S=2048, D=1024, H=16) on 8 NeuronCores.

Sharding: core c handles batch b = c//4 and 4 heads starting at (c%4)*4
(data parallel over batch x tensor parallel over heads; wQ/wK/wV split
column-wise by head, wO row-wise; partial outputs summed on host).

v2: single fused pipeline, engines balanced against the cost model:
  - Projections in fp8 hi-lo DoubleRow form: x = x_hi + x_lo and w*64 =
    w_hi + w_lo (both fp8e4, host-prepared; the *64 keeps w out of fp8's
    subnormal range), accumulate x_hi*w_hi + x_hi*w_lo + x_lo*w_hi in psum
    (the dropped lo*lo term is ~2^-8 relative), scale 1/64 + bias applied at
    copyback. DoubleRow contracts 256 model-dims per matmul at 0.5 cyc/row,
    so a projection costs 0.75x its bf16 form in PE time at bf16-level
    accuracy.
  - Projections stream in 512-query-column chunks interleaved with the mask
    DMAs so the PE starts ~4us in: Q[0:1024] first (ACT copyback while ACT is
    idle), then K, V, Q[1024:2048] (DVE copybacks once the exp stream owns
    ACT).
  - Attention per (s1c query-1024-block, head): scoresT psum [128,1024]
    (2 bf16 matmuls), exp on ACT (psum -> sbuf bf16), {0,1}-mask multiply on
    DVE (bf16 2x mode), PV into two [128,512] psum chains. Each head's V4x
    block is [64 ones-columns | 64 V-columns], so the PV matmul lands the
    softmax denominator REPLICATED on psum partitions 0-63 and ctx on
    64-127 (out-free-size-only matmul cost makes the wide ones block free).
  - Normalization per chain: one 64-wide reciprocal_approx_fast at partition
    base 0 (custom-DVE ops only honor base 0 on real HW) straight from psum,
    then one multiply (mixed 32-aligned operand bases are legal). Odd heads
    restack to partitions 64-127 via a small batched SBUF-to-SBUF DMA.
  - PV for kc is emitted D kc-steps late so the PE queue never head-blocks
    on the exp chain of the same kc; the DMA issue order is hand-sequenced
    (K fully first: exp needs only scores, so the exp stream unblocks early
    while mask-muls and PV lag in the et pool until masks/V land).
  - Output projection per query half into [128,512] psums; copyback on DVE
    for s1c=0 (ACT is exp-saturated), alternating DVE/ACT in the tail for
    s1c=1; 256-row quarters ship as single batched DMAs. s1c=1 processes
    heads 0,1,3,2 so the final head needs no restack DMA on the tail path.
  - PSUM: scores 2x[128,1024] (4 banks) + PV 2x[128,512] (2) + proj/outproj
    2x[128,512] (2) = 8 banks; the tail outproj also borrows the idle
    scores pool for a 4-deep ring.

The wV bias is dropped on device: softmax rows sum to 1, so it contributes
exactly wV_b @ wO_w.T, folded into the host-side bias add. No max-subtraction
in softmax (scores are O(5), bf16 exp cannot overflow).

Host: out[b] = sum of the 4 cores' partials + (wO_b + wV_b @ wO_w.T).
"""

import numpy as np
from contextlib import ExitStack

import concourse.bacc as bacc
import concourse.tile as tile
from concourse import mybir
import ml_dtypes

B, S, DM, H, DK = 2, 2048, 1024, 16, 64
NCORES = 8
GROUPS = 4          # cores per batch
HPC = H // GROUPS   # heads per core = 4
P = 128
KT = DM // P        # 8 k-tiles over the model dim
CW = HPC * DK       # projected width per core = 256
SCALE = 1.0 / np.sqrt(DK)
WS = 64.0           # fp8 weight pre-scale (power of 2; undone at copyback)

MM_DT = mybir.dt.bfloat16
MM_NP = ml_dtypes.bfloat16
F8_DT = mybir.dt.float8e4
F8_NP = ml_dtypes.float8_e4m3

F32 = mybir.dt.float32
EXPF = mybir.ActivationFunctionType.Exp
IDENT = mybir.ActivationFunctionType.Identity
DR = mybir.MatmulPerfMode.DoubleRow

_cache: dict = {}


def _build():
    nc = bacc.Bacc("TRN2", target_bir_lowering=False, debug=False)

    xs = {}
    for nm in ("q", "k", "v"):
        for part in ("hi", "lo"):
            xs[(nm, part)] = nc.dram_tensor(f"x{nm}_{part}", [DM, S], F8_DT,
                                            kind="ExternalInput")
    ws = {}
    for nm in ("q", "k", "v"):
        for part in ("hi", "lo"):
            ws[(nm, part)] = nc.dram_tensor(f"w{nm}_{part}", [DM, CW], F8_DT,
                                            kind="ExternalInput")
    wqb = nc.dram_tensor("wqb", [CW, 1], F32, kind="ExternalInput")
    wkb = nc.dram_tensor("wkb", [CW, 1], F32, kind="ExternalInput")
    woT = nc.dram_tensor("woT", [CW, DM], MM_DT, kind="ExternalInput")
    maskT = nc.dram_tensor("maskT", [S, S], MM_DT, kind="ExternalInput")
    out = nc.dram_tensor("out", [S, DM], MM_DT, kind="ExternalOutput")

    with tile.TileContext(nc) as tc, ExitStack() as ctx:
        const = ctx.enter_context(tc.tile_pool(name="const", bufs=1))
        big = ctx.enter_context(tc.tile_pool(name="big", bufs=1))
        xpool = ctx.enter_context(tc.tile_pool(name="xpool", bufs=2))
        mpool = ctx.enter_context(tc.tile_pool(name="mpool", bufs=1))
        epool = ctx.enter_context(tc.tile_pool(name="epool", bufs=21))
        rpool = ctx.enter_context(tc.tile_pool(name="rpool", bufs=2))
        opool = ctx.enter_context(tc.tile_pool(name="opool", bufs=3))
        pp_sc = ctx.enter_context(tc.tile_pool(name="pp_sc", bufs=2, space="PSUM"))
        pp_ctx = ctx.enter_context(tc.tile_pool(name="pp_ctx", bufs=2, space="PSUM"))
        pp_pj = ctx.enter_context(tc.tile_pool(name="pp_pj", bufs=2, space="PSUM"))

        # persistent activations
        # Q4T/K4T: [dk-in-pair (part), pair c, s]; head h = c*2 + (p//64)
        Q4T = big.tile([P, 2, S], MM_DT, name="Q4T")
        K4T = big.tile([P, 2, S], MM_DT, name="K4T")
        V4x = big.tile([P, 16, HPC * 2 * DK], MM_DT, name="V4x")
        ctxT = [[big.tile([P, 1024], MM_DT, name=f"ctxT{i}_{j}") for j in range(2)]
                for i in range(2)]

        # 64 ones columns per head (cols h*128..h*128+63): the PV matmul then
        # lands the softmax denominator replicated on psum partitions 0-63
        # (reciprocal_approx_fast only honors partition base 0 on HW), and
        # ctx on partitions 64-127.
        for h in range(HPC):
            nc.vector.memset(V4x[:, :, h * 128 : h * 128 + 64], 1.0)

        w_sbs = {}
        b_sbs = {}

        def load_w_part(nm, part):
            w_sb = const.tile([P, KT, CW], F8_DT, name=f"w{nm}{part}_sb")
            nc.sync.dma_start(
                out=w_sb[:],
                in_=ws[(nm, part)].ap().rearrange("(t p) m -> p t m", p=P))
            w_sbs[(nm, part)] = w_sb

        def load_w(nm, bd):
            for part in ("hi", "lo"):
                load_w_part(nm, part)
            if bd is not None:
                b_sb = const.tile([P, 2], F32, name=f"b{nm}_sb")
                nc.sync.dma_start(
                    out=b_sb[:], in_=bd.ap().rearrange("(m p) o -> p (m o)", p=P))
                b_sbs[nm] = b_sb

        def load_bias(nm, bd):
            b_sb = const.tile([P, 2], F32, name=f"b{nm}_sb")
            nc.sync.dma_start(
                out=b_sb[:], in_=bd.ap().rearrange("(m p) o -> p (m o)", p=P))
            b_sbs[nm] = b_sb

        x_tiles = {}
        def load_x_part(nm, sc, part, bufs=2):
            t = xpool.tile([P, KT, 512], F8_DT, tag=f"x{nm}{part}", bufs=bufs)
            nc.sync.dma_start(
                out=t[:],
                in_=xs[(nm, part)].ap().rearrange("(t p) s -> p t s", p=P)
                [:, :, sc * 512:(sc + 1) * 512])
            x_tiles.setdefault((nm, sc), [None, None])[0 if part == "hi" else 1] = t

        def load_x(nm, sc, bufs=2):
            """Issue DMA for x columns [sc*512, (sc+1)*512] (hi+lo)."""
            for part in ("hi", "lo"):
                load_x_part(nm, sc, part, bufs=bufs)

        def proj_qk(nm, dst, sc, eng):
            """Project x cols [sc*512,(sc+1)*512] -> dst[:, :, cols] (+bias)."""
            xh, xl = x_tiles.pop((nm, sc))
            wh, wl = w_sbs[(nm, "hi")], w_sbs[(nm, "lo")]
            b_sb = b_sbs[nm]
            for m in range(2):
                ps = pp_pj.tile([P, 512], F32, tag="pj", name="ps_pj")
                i = 0
                for wt, xt in ((wh, xh), (wh, xl), (wl, xh)):
                    for c in range(4):
                        nc.tensor.matmul(
                            ps[:], wt[:, 2 * c:2 * c + 2, m * 128:(m + 1) * 128],
                            xt[:, 2 * c:2 * c + 2, :],
                            start=(i == 0), stop=(i == 11), perf_mode=DR)
                        i += 1
                dcols = dst[:, m, sc * 512:(sc + 1) * 512]
                if eng == "act":
                    nc.scalar.activation(dcols, ps[:], IDENT,
                                         bias=b_sb[:, m:m + 1], scale=1.0 / WS)
                else:
                    nc.vector.tensor_scalar(
                        out=dcols, in0=ps[:], scalar1=1.0 / WS,
                        scalar2=b_sb[:, m:m + 1],
                        op0=mybir.AluOpType.mult, op1=mybir.AluOpType.add)

        def proj_v(sc):
            """Project v cols [sc*512,(sc+1)*512] (4 s-tiles of 128)."""
            xh, xl = x_tiles.pop(("v", sc))
            wh, wl = w_sbs[("v", "hi")], w_sbs[("v", "lo")]
            for st in range(4):
                ps = pp_pj.tile([P, 512], F32, tag="pj", name="ps_pj")
                i = 0
                for xt, wt in ((xh, wh), (xh, wl), (xl, wh)):
                    for c in range(4):
                        nc.tensor.matmul(
                            ps[:, 0:CW],
                            xt[:, 2 * c:2 * c + 2, st * 128:(st + 1) * 128],
                            wt[:, 2 * c:2 * c + 2, :],
                            start=(i == 0), stop=(i == 11), perf_mode=DR)
                        i += 1
                sidx = sc * 4 + st
                nc.vector.tensor_scalar(
                    out=V4x.rearrange("p s (h e) -> p s h e", e=2 * DK)[:, sidx, :, DK:2 * DK],
                    in0=ps[:, 0:CW].rearrange("p (h e) -> p h e", e=DK),
                    scalar1=1.0 / WS, scalar2=None, op0=mybir.AluOpType.mult)

        # mask chunks: [(s1c, mc)] covers kc in [mc*4, mc*4+4) for query block s1c
        mask_sb = {}
        def load_mask(s1c, mc):
            mt = mpool.tile([P, 4, 1024], MM_DT, tag=f"m{mc}")
            nc.sync.dma_start(
                out=mt[:],
                in_=maskT.ap().rearrange("(t p) s -> p t s", p=P)
                [:, mc * 4:(mc + 1) * 4, s1c * 1024:(s1c + 1) * 1024])
            mask_sb[(s1c, mc)] = mt

        # ---------------- lead-in: DMA issue order + Q/K projections --------
        # DMA device is serialized; this issue order is the delivery order.
        # K ships first and completely: exp needs only scores (K x Q), so the
        # whole exp stream unblocks early; mask-muls and PV lag behind in the
        # et pool until the masks/V land.
        load_w_part("k", "hi")
        load_x_part("k", 0, "hi", bufs=3)
        load_x_part("k", 0, "lo", bufs=3)
        load_w_part("k", "lo")
        load_bias("k", wkb)
        load_w_part("q", "hi")
        load_x_part("q", 0, "hi")
        load_x_part("q", 0, "lo")
        load_w_part("q", "lo")
        load_bias("q", wqb)
        load_x("q", 1)
        load_x("k", 1, bufs=3)
        load_x("k", 2, bufs=3)
        load_x("k", 3, bufs=3)
        load_w("v", None)
        load_mask(0, 0)
        load_x("v", 0)
        load_mask(0, 1)
        load_x("v", 1)
        load_mask(0, 2)
        load_x("v", 2)
        load_mask(0, 3)
        load_x("v", 3)
        woT_sb = const.tile([P, 2, DM], MM_DT, name="woT_sb")

        proj_qk("k", K4T, 0, "act")
        proj_qk("q", Q4T, 0, "act")
        proj_qk("q", Q4T, 1, "act")
        for sc in range(1, 4):
            proj_qk("k", K4T, sc, "dve")

        # ---------------- attention ----------------
        def outproj(s1c, qh, eng, pools=None, only_sb=None):
            """Output projection for query rows [s1c*1024 + qh*512 ...+512].

            Copybacks collect into [P, 2, 2, 512] staging tiles so each
            256-row quarter ships as a single DMA (HWDGE issue overhead
            dominates these short transfers). `pools` can alternate psum
            pools (the scores pool is idle in the tail)."""
            if pools is None:
                pools = [(pp_pj, "pj")]
            pi = 0
            for sb in ((0, 1) if only_sb is None else (only_sb,)):
                ob = opool.tile([P, 2, 2, 512], MM_DT, tag="ob")
                for st in range(2):
                    q0 = qh * 512 + (sb * 2 + st) * 128
                    for n2 in range(2):
                        pool, ptag = pools[pi % len(pools)]
                        pi += 1
                        ps = pool.tile([P, 512], F32, tag=ptag, name="ps_pj")
                        for c2 in range(2):
                            nc.tensor.matmul(
                                ps[:], ctxT[c2][s1c][:, q0:q0 + 128],
                                woT_sb[:, c2, n2 * 512:(n2 + 1) * 512],
                                start=(c2 == 0), stop=(c2 == 1))
                        e = eng if eng != "mix" else ("act" if (st + n2) % 2 else "dve")
                        if e == "act":
                            nc.scalar.activation(ob[:, st, n2, :], ps[:], IDENT)
                        else:
                            nc.vector.tensor_copy(out=ob[:, st, n2, :], in_=ps[:])
                row = s1c * 1024 + qh * 512 + sb * 256
                nc.sync.dma_start(
                    out=out.ap()[row:row + 256, :]
                    .rearrange("(t p) (n2 n) -> p t n2 n", p=P, n2=2),
                    in_=ob[:])

        def head(s1c, h, extra):
            """One head's scoresT/exp/mask/PV stream for query block s1c.

            `extra` maps kc -> thunk emitted after that kc's scores/exp/mask
            (projection or outproj work interleaved into this head's stream).
            PV for kc is emitted D steps late so the PE queue never
            head-blocks on the exp+mask chain of the same kc.
            """
            D = 5 if extra else 3
            base = (h % 2) * 64
            c = h // 2
            cps = [pp_ctx.tile([P, 512], F32, tag="cps", name=f"cps{i}")
                   for i in range(2)]

            def pv(kc):
                for qh in range(2):
                    nc.tensor.matmul(
                        cps[qh][:], V4x[:, kc, h * 128:(h + 1) * 128],
                        ets[kc][:, qh * 512:(qh + 1) * 512],
                        start=(kc == 0), stop=(kc == 15))

            ets = {}
            for kc in range(16):
                # PV first: it is ready (its et landed D tiles ago), so it
                # fills the PE queue while the scores matmul below waits for
                # its psum buffer to be drained by ACT.
                if kc >= D:
                    pv(kc - D)
                ps = pp_sc.tile([P, 1024], F32, tag="sc", name="ps_sc")
                lhsT = K4T[base:base + 64, c, kc * 128:(kc + 1) * 128]
                for n2 in range(2):
                    col = s1c * 1024 + n2 * 512
                    nc.tensor.matmul(
                        ps[:, n2 * 512:(n2 + 1) * 512], lhsT,
                        Q4T[base:base + 64, c, col:col + 512],
                        start=True, stop=True)
                et = epool.tile([P, 1024], MM_DT, tag="et")
                nc.scalar.activation(et[:], ps[:], EXPF)
                nc.vector.tensor_mul(
                    et[:], et[:], mask_sb[(s1c, kc // 4)][:, kc % 4, :])
                ets[kc] = et
                if kc in extra:
                    extra[kc]()
            for kc in range(16 - D, 16):
                pv(kc)
            ht = None if h % 2 == 0 else rpool.tile([64, 1024], MM_DT, tag="ht")
            for qh in range(2):
                # 64-wide reciprocal of the replicated rowsum at partition
                # base 0 (custom-DVE ops only honor base 0 on HW), then one
                # multiply against ctx on partitions 64-127
                rc = rpool.tile([64, 512], F32, tag="rc")
                nc.vector.reciprocal_approx_fast(out=rc[:], in_=cps[qh][0:64, :])
                qcols = slice(qh * 512, (qh + 1) * 512)
                if h % 2 == 0:
                    nc.vector.tensor_mul(
                        ctxT[c][s1c][0:64, qcols], cps[qh][64:128, :], rc[:])
                else:
                    nc.vector.tensor_mul(
                        ht[:, qcols], cps[qh][64:128, :], rc[:])
            if h % 2 == 1:
                nc.sync.dma_start(out=ctxT[c][s1c][64:128, :], in_=ht[:])

        # s1c=0: V projections stream through head 0 (chunk j unlocks PV for
        # kc in [4j, 4j+4)); remaining DMAs (xv2/3, xq2/3, s1c=1 masks, woT)
        # are issued from inside the head stream so the serialized DMA device
        # delivers in need-order; Q[1024:2048] projections ride heads 2-3.
        head(0, 0, {3: lambda: proj_v(0),
                    7: lambda: proj_v(1),
                    11: lambda: proj_v(2),
                    15: lambda: proj_v(3)})
        head(0, 1, {1: lambda: load_x("q", 2),
                    5: lambda: load_x("q", 3)})
        head(0, 2, {7: lambda: proj_qk("q", Q4T, 2, "dve")})
        head(0, 3, {5: lambda: load_mask(1, 0),
                    7: lambda: proj_qk("q", Q4T, 3, "dve"),
                    9: lambda: load_mask(1, 1),
                    11: lambda: nc.sync.dma_start(
                        out=woT_sb[:],
                        in_=woT.ap().rearrange("(c p) n -> p c n", p=P)),
                    13: lambda: load_mask(1, 2)})
        # s1c=1: outproj for s1c=0 rides heads 0-1; s1c=1's own outproj
        # lands in the tail where ACT is free.
        head(1, 0, {1: lambda: load_mask(1, 3),
                    5: lambda: outproj(0, 0, "dve", only_sb=0)})
        head(1, 1, {5: lambda: outproj(0, 0, "dve", only_sb=1)})
        head(1, 3, {5: lambda: outproj(0, 1, "dve", only_sb=0)})
        head(1, 2, {5: lambda: outproj(0, 1, "dve", only_sb=1)})
        tailpools = [(pp_pj, "pj"), (pp_sc, "sc")]
        outproj(1, 0, "mix", pools=tailpools)
        outproj(1, 1, "mix", pools=tailpools)

    nc.compile()
    return nc


def get_nc():
    if "nc" not in _cache:
        _cache["nc"] = _build()
    return _cache["nc"]


def make_in_maps(q, k, v, mask, wQ_w, wQ_b, wK_w, wK_b, wV_w, wV_b, wO_w, wO_b):
    q = np.asarray(q, np.float32)
    k = np.asarray(k, np.float32)
    v = np.asarray(v, np.float32)
    mask = np.asarray(mask)

    def hilo(a):
        hi = a.astype(F8_NP)
        lo = (a - hi.astype(np.float32)).astype(F8_NP)
        return hi, lo

    xq_hi, xq_lo = hilo(np.ascontiguousarray(q.transpose(0, 2, 1)))
    xk_hi, xk_lo = hilo(np.ascontiguousarray(k.transpose(0, 2, 1)))
    xv_hi, xv_lo = hilo(np.ascontiguousarray(v.transpose(0, 2, 1)))
    mT = np.ascontiguousarray(mask[:, 0].transpose(0, 2, 1)).astype(MM_NP)
    in_maps = []
    for c in range(NCORES):
        b = c // GROUPS
        rows = slice((c % GROUPS) * HPC * DK, ((c % GROUPS) + 1) * HPC * DK)
        wqs = np.ascontiguousarray(np.asarray(wQ_w, np.float32)[rows].T) \
            * np.float32(SCALE * WS)
        wks = np.ascontiguousarray(np.asarray(wK_w, np.float32)[rows].T) \
            * np.float32(WS)
        wvs = np.ascontiguousarray(np.asarray(wV_w, np.float32)[rows].T) \
            * np.float32(WS)
        m = {
            "xq_hi": xq_hi[b], "xq_lo": xq_lo[b],
            "xk_hi": xk_hi[b], "xk_lo": xk_lo[b],
            "xv_hi": xv_hi[b], "xv_lo": xv_lo[b],
            "wqb": (np.asarray(wQ_b, np.float32)[rows] * np.float32(SCALE)).reshape(-1, 1),
            "wkb": np.asarray(wK_b, np.float32)[rows].reshape(-1, 1),
            "woT": np.ascontiguousarray(np.asarray(wO_w, np.float32)[:, rows].T).astype(MM_NP),
            "maskT": mT[b],
        }
        for nm, warr in (("q", wqs), ("k", wks), ("v", wvs)):
            hi, lo = hilo(warr)
            m[f"w{nm}_hi"] = hi
            m[f"w{nm}_lo"] = lo
        in_maps.append(m)
    return in_maps


def _get_runner():
    """Cached jitted 8-core runner (one XLA/walrus compile per process)."""
    if "runner" in _cache:
        return _cache["runner"]
    import jax
    from jax.sharding import Mesh, PartitionSpec, NamedSharding
    from jax.experimental.shard_map import shard_map
    from concourse.bass2jax import (
        _bass_exec_p, install_neuronx_cc_hook, partition_id_tensor)

    nc = get_nc()
    install_neuronx_cc_hook()
    pname = nc.partition_id_tensor.name if nc.partition_id_tensor else None
    in_names, out_names, out_avals = [], [], []
    for alloc in nc.m.functions[0].allocations:
        if not isinstance(alloc, mybir.MemoryLocationSet):
            continue
        name = alloc.memorylocations[0].name
        if alloc.kind == "ExternalInput":
            if name != pname:
                in_names.append(name)
        elif alloc.kind == "ExternalOutput":
            out_names.append(name)
            out_avals.append(jax.core.ShapedArray(
                tuple(alloc.tensor_shape), mybir.dt.np(alloc.dtype)))
    n_params = len(in_names)
    all_names = in_names + out_names
    if pname is not None:
        all_names = all_names + [pname]

    def _body(*args):
        operands = list(args)
        if pname is not None:
            operands.append(partition_id_tensor())
        outs = _bass_exec_p.bind(
            *operands,
            out_avals=tuple(out_avals),
            in_names=tuple(all_names),
            out_names=tuple(out_names),
            lowering_input_output_aliases=(),
            sim_require_finite=True,
            sim_require_nnan=True,
            nc=nc,
        )
        return tuple(outs)

    devices = jax.devices()[:NCORES]
    mesh = Mesh(np.asarray(devices), ("core",))
    nin = n_params + len(out_names)
    fn = jax.jit(shard_map(
        _body, mesh=mesh,
        in_specs=(PartitionSpec("core"),) * nin,
        out_specs=(PartitionSpec("core"),) * len(out_names),
        check_rep=False,
    ), keep_unused=True)
    sharding = NamedSharding(mesh, PartitionSpec("core"))
    zeros = [np.zeros((NCORES * a.shape[0], *a.shape[1:]), a.dtype)
             for a in out_avals]

    def run(in_maps):
        concat = [np.concatenate([np.asarray(m[n]) for m in in_maps], axis=0)
                  for n in in_names]
        args = [jax.device_put(x, sharding) for x in concat + zeros]
        outs = fn(*args)
        o = np.asarray(outs[0]).reshape(NCORES, S, DM)
        return [o[c] for c in range(NCORES)]

    _cache["runner"] = run
    return run


def kernel(q, k, v, mask, wQ_w, wQ_b, wK_w, wK_b, wV_w, wV_b, wO_w, wO_b):
    run = _get_runner()
    in_maps = make_in_maps(q, k, v, mask, wQ_w, wQ_b, wK_w, wK_b, wV_w, wV_b,
                           wO_w, wO_b)
    outs = run(in_maps)
    ob = (np.asarray(wO_b, np.float64)
          + np.asarray(wV_b, np.float64) @ np.asarray(wO_w, np.float64).T).astype(np.float32)
    full = np.empty((B, S, DM), np.float32)
    for b in range(B):
        acc = outs[b * GROUPS].astype(np.float32)
        for g in range(1, GROUPS):
            acc = acc + outs[b * GROUPS + g]
        full[b] = acc + ob[None, :]
    return full
